# revision 1
# baseline (speedup 1.0000x reference)
"""Causal depthwise conv1d (K=4) + SiLU on TRN2 — channel-major fp16 design.

Device kernel per core (R=2048 out rows, D=2048 channels):

    DMA in (fp16, channel-major strips with per-channel weights embedded) ->
    K=4 accumulating diagonal matmuls per (d-block, l-chunk) on the PE
    (stationary = diag(w_k) fp16, moving = shifted strip slice, fp32 PSUM) ->
    ACT Silu (PSUM -> SBUF fp16) -> DMA out (fp16, channel-major)

The host pre-transposes each shard to channel-major fp16 (with K-1 halo
cols and the 4 weight taps prepended to every channel row) and
un-transposes/upcasts the output on gather. rel err ~1e-3 << 2e-2 gate.

Hardware facts learned from NTFF traces (drove this design):
 - PE runs the conv at 1 col/cycle (215 ns per 512-col matmul, LDWEIGHTS
   fully hidden): 55 us steady-state is this algorithm's floor.
 - DVE/GpSimd elementwise offload of conv blocks is 8-25x slower than
   nominal rates -> everything stays on PE.
 - A DMA into a [128, *] tile costs ~28 ns/descriptor * 128 descriptors
   ~= 3.6 us of queue-serial time REGARDLESS of bytes; queue throughput
   = bytes-per-partition/28ns. Hence: partition-major group tensors
   (one descriptor spans several blocks per partition), block 0 split
   across two queues by partition halves, tail groups LAST in the sync
   queue's FIFO (so they cannot steal early bandwidth), outputs
   alternating between the gpsimd and scalar queues, last output split
   by partition halves. Queue FIFO order is the only reliable pacing
   tool -- the tile scheduler reorders engine streams, and 3-way
   partition splits / input DMAs on the gpsimd queue both regressed.
 - Block 0's diag matrices ride in g0 host-prebuilt (first 4*128 cols),
   so the very first matmul waits only on the g0 DMAs, not on a DVE
   diag-build hop (-1.2 us). Later blocks' diag builds are off the
   critical path (the PE stream is gapless), so they stay on DVE.
"""

from contextlib import ExitStack

import numpy as np

import concourse.bass as bass
import concourse.mybir as mybir
import concourse.tile as tile
from concourse.masks import make_identity

F16 = mybir.dt.float16
F32 = mybir.dt.float32
SILU = mybir.ActivationFunctionType.Silu
MULT = mybir.AluOpType.mult

# Input groups: (first block, #blocks). Group 0 is block 0 (split by
# partition halves over two queues for the earliest possible start);
# the last group is deferred and lands while blocks 0-7 compute.
GROUPS = ((0, 1), (1, 2), (3, 3), (6, 2), (8, 4), (12, 4))


def build_conv_kernel(
    nc: bass.Bass,
    R: int,            # output rows (l) per core
    D: int,            # channels (multiple of 128)
    K: int = 4,
    L_CHUNK: int = 512,
    pc_bufs: int = 8,
    ot_bufs: int = 4,
):
    HALO = K - 1
    NB = D // 128            # d-blocks of 128 channels
    RS = R + HALO            # strip length (halo prepended)
    WC = 2 * K               # fp32 weights bitcast into 2K fp16 cols
    ROW = WC + RS + 1        # per-channel row: [w fp32 x4 | halo | x | pad]
    assert ROW % 2 == 0      # even so the fp32 bitcast view works
    NCH = R // L_CHUNK       # l-chunks per block
    assert R % L_CHUNK == 0 and D % 128 == 0

    # group 0 carries block 0's four host-prebuilt diag matrices in its
    # first 4*128 cols: the PE's first matmul then waits only on the g0
    # DMAs, not on a DVE diag-build hop
    DG0 = K * 128
    g_d = [nc.dram_tensor(f"g{i}", [128, n * ROW + (DG0 if i == 0 else 0)],
                          F16, kind="ExternalInput")
           for i, (_, n) in enumerate(GROUPS)]
    o_d = nc.dram_tensor("out", [D, R], F16, kind="ExternalOutput")

    with ExitStack() as ctx:
        tc = ctx.enter_context(tile.TileContext(nc))

        const_pool = ctx.enter_context(tc.tile_pool(name="const", bufs=1))
        xt_pool = ctx.enter_context(tc.tile_pool(name="xt", bufs=1))
        ot_pool = ctx.enter_context(tc.tile_pool(name="ot", bufs=ot_bufs))
        pc_pool = ctx.enter_context(tc.tile_pool(name="pc", bufs=pc_bufs,
                                                 space="PSUM"))

        # Input group tiles. Group 0 (block 0): two partition-half DMAs on
        # two queues (64 descriptors each, earliest possible first conv).
        # Groups 1-5 in FIFO order on sync: ascending by need-time, tail
        # groups last so they can't steal early DMA bandwidth.
        g_tiles = []
        for i, (gb, gn) in enumerate(GROUPS):
            t = xt_pool.tile([128, gn * ROW + (DG0 if i == 0 else 0)],
                             F16, name=f"g{i}")
            g_tiles.append(t)
        nc.sync.dma_start(g_tiles[0][0:64, :], g_d[0][0:64, :])
        nc.scalar.dma_start(g_tiles[0][64:128, :], g_d[0][64:128, :])
        # sync's queue is FIFO: the tail groups (blocks 8-15) are LAST,
        # so they can't steal DMA bandwidth from the critical early blocks
        for i in (1, 2, 3, 4, 5):
            nc.sync.dma_start(g_tiles[i], g_d[i][:, :])

        # block -> (group tile, column base of that block's row)
        src = {}
        for i, (gb, gn) in enumerate(GROUPS):
            for j in range(gn):
                src[gb + j] = (g_tiles[i], j * ROW + (DG0 if i == 0 else 0))

        ident = const_pool.tile([128, 128], F32)
        make_identity(nc, ident)
        ident16 = const_pool.tile([128, 128], F16)
        nc.vector.tensor_copy(ident16, ident)

        # diag(w[:, b, k]) fp16, one broadcast DVE instr per block:
        # diags3[p, b*K+k, f] = ident[p, f] * w_b[p, k]
        diags = const_pool.tile([128, NB * K * 128], F16)
        diags3 = diags.rearrange("p (c f) -> p c f", c=NB * K)
        ibc = ident16.rearrange("p (c f) -> p c f", c=1).broadcast_to(
            [128, K, 128])

        def diag_tt(b):
            t, base = src[b]
            nc.vector.tensor_tensor(
                diags3[:, b * K:(b + 1) * K, :],
                ibc,
                t[:, base:base + WC].bitcast(F32).rearrange(
                    "p (c f) -> p c f", f=1).broadcast_to([128, K, 128]),
                MULT,
            )

        for b in range(1, NB):     # block 0's diags came in with g0
            diag_tt(b)

        for b in range(NB):
            ot = ot_pool.tile([128, R], F16, tag="ot")
            t, base = src[b]
            last = b == NB - 1
            for c in range(NCH):
                pc = pc_pool.tile([128, L_CHUNK], F32, tag="pc")
                for k in range(K):
                    stat = (g_tiles[0][:, k * 128:(k + 1) * 128] if b == 0
                            else diags[:, (b * K + k) * 128:
                                       (b * K + k + 1) * 128])
                    nc.tensor.matmul(
                        pc,
                        stat,
                        t[:, base + WC + c * L_CHUNK + k:
                          base + WC + c * L_CHUNK + k + L_CHUNK],
                        start=(k == 0),
                        stop=(k == K - 1),
                    )
                nc.scalar.activation(ot[:, c * L_CHUNK:(c + 1) * L_CHUNK],
                                     pc, SILU)
                if last and c % 2 == 1:
                    # last block: ship each finished column-half right
                    # away, split by partition halves over both idle
                    # queues -- the first pair's descriptor time overlaps
                    # the final convs, shortening the tail
                    c0 = (c - 1) * L_CHUNK
                    c1 = (c + 1) * L_CHUNK
                    nc.gpsimd.dma_start(
                        o_d[b * 128:b * 128 + 64, c0:c1], ot[0:64, c0:c1])
                    nc.scalar.dma_start(
                        o_d[b * 128 + 64:(b + 1) * 128, c0:c1],
                        ot[64:128, c0:c1])
            if last:
                pass
            elif b % 2 == 0:
                nc.gpsimd.dma_start(o_d[b * 128:(b + 1) * 128, :], ot)
            else:
                nc.scalar.dma_start(o_d[b * 128:(b + 1) * 128, :], ot)

    return nc


# ---------------------------------------------------------------------------
# Entry point: full (unsharded) inputs -> full output, 8 NeuronCores.
# ---------------------------------------------------------------------------
from concourse.bass_utils import run_bass_kernel_spmd
import concourse.bacc as bacc

_B, _L, _D, _K = 4, 4096, 2048, 4
_N_CORES = 8
_SHARDS_PER_BATCH = _N_CORES // _B
_LC = _L // _SHARDS_PER_BATCH     # 2048 output rows per core
_HALO = _K - 1
_ROW = 2 * _K + _LC + _HALO + 1   # [w fp32 x4 (bitcast) | halo | x | pad]
_NB = _D // 128

TRACE = False
LAST_EXEC_TIME_NS = None

_compiled_nc = None


def _get_nc():
    global _compiled_nc
    if _compiled_nc is None:
        nc = bacc.Bacc("TRN2", target_bir_lowering=False, debug=False)
        build_conv_kernel(nc, _LC, _D, K=_K, L_CHUNK=512)
        nc.compile()
        _compiled_nc = nc
    return _compiled_nc


def kernel(inputs: np.ndarray, weight: np.ndarray) -> np.ndarray:
    """inputs: (4, 4096, 2048) fp32; weight: (2048, 1, 4) fp32.

    Returns silu(causal_depthwise_conv1d(inputs, weight)): (4, 4096, 2048).
    Sharding: data parallel over (batch, L-chunk); each core's shard is
    pre-transposed to channel-major fp16 (weights + halo prepended per
    channel row) and regrouped partition-major per DMA group host-side.
    """
    global LAST_EXEC_TIME_NS
    x_full = np.asarray(inputs, dtype=np.float32)
    w_full = np.asarray(weight, dtype=np.float32)
    assert x_full.shape == (_B, _L, _D), x_full.shape

    # fp32 weights bitcast into pairs of fp16 slots per channel row
    w32 = np.ascontiguousarray(w_full.reshape(_D, _K).astype(np.float32))
    w_as16 = w32.view(np.float16)                      # [d, 2K]

    in_maps = []
    for c in range(_N_CORES):
        b, s = divmod(c, _SHARDS_PER_BATCH)
        l0 = s * _LC
        strip = np.empty((_D, _ROW), dtype=np.float16)
        strip[:, :2 * _K] = w_as16
        if s == 0:
            strip[:, 2 * _K:2 * _K + _HALO] = 0.0
        else:
            strip[:, 2 * _K:2 * _K + _HALO] = x_full[b, l0 - _HALO:l0].T
        strip[:, 2 * _K + _HALO:-1] = x_full[b, l0:l0 + _LC].T
        strip[:, -1] = 0.0
        # partition-major group tensors: g[p, j*ROW:(j+1)*ROW] is the row
        # of channel (gb+j)*128 + p
        s3 = strip.reshape(_NB, 128, _ROW)
        im = {}
        for i, (gb, gn) in enumerate(GROUPS):
            g = s3[gb:gb + gn].transpose(1, 0, 2).reshape(128, gn * _ROW)
            if i == 0:
                # prepend block 0's diag matrices: d0[p, k*128+f]
                #   = w[p, k] * (p == f)
                wk0 = w_full.reshape(_D, _K)[:128].astype(np.float16)
                eye = np.eye(128, dtype=np.float16)
                d0 = (wk0[:, :, None] * eye[:, None, :]).reshape(128, -1)
                g = np.concatenate([d0, g], axis=1)
            im[f"g{i}"] = np.ascontiguousarray(g)
        in_maps.append(im)

    nc = _get_nc()
    res = run_bass_kernel_spmd(nc, in_maps, list(range(_N_CORES)),
                               trace=TRACE)
    LAST_EXEC_TIME_NS = res.exec_time_ns

    out = np.empty((_B, _L, _D), dtype=np.float32)
    for c in range(_N_CORES):
        b, s = divmod(c, _SHARDS_PER_BATCH)
        out[b, s * _LC:(s + 1) * _LC] = res.results[c]["out"].T.astype(
            np.float32)
    return out



# revision 2
# speedup vs baseline: 1.3299x; 1.3299x over previous
"""Causal depthwise conv1d (K=4) + SiLU on TRN2 — int8 / time-phase-packed design.

Per core (R=2048 out rows, D=2048 channels), v2 design:

 - Host quantizes each shard to int8 with per-channel scales (absmax/127,
   halo-aware; rel err ~7e-3 << 2e-2 gate) and packs it time-phase-major:
   partition p = (c, i) = 32 channels x 4 time phases, column t = coarse
   time block of 4 samples, one halo column per 32-channel block.
 - DMA in on gpsimd (SWDGE) casts int8 -> fp16 inline: HBM input bytes
   halve to 4.2 MB/core.
 - PE: per 32-channel block, TWO accumulating matmuls (main + carry)
   of 512 cols each compute all 4 taps: stationary embeds the 4x4
   time-phase kernel per channel on the block diagonal. 64 blocks x 2
   x 512 cols = 65536 PE cycles ~ 27.5 us warm (vs 55 us for the
   4-matmul diagonal scheme).
 - Stationaries are built on DVE: one broadcast multiply per 8-block
   chunk: chandiag (c==c' mask) x per-(p,j) weight tile (w*scale
   folded in, shipped from host, ~1KB).
 - ACT: Silu over [128, 2048] fp32 PSUM reads (4 banks) -> fp16 SBUF:
   16 instrs ~ 30 us total (vs 44 us at 512 cols).
 - DMA out fp16 on sync/scalar (HWDGE) alternating, 512KB per DMA.

HBM bytes/core: 4.2 in + 8.4 out = 12.7 MB -> ~32-40 us DMA-bound floor.
"""

from contextlib import ExitStack

import numpy as np

import concourse.bass as bass
import concourse.mybir as mybir
import concourse.tile as tile

F16 = mybir.dt.float16
F32 = mybir.dt.float32
I8 = mybir.dt.int8
SILU = mybir.ActivationFunctionType.Silu
MULT = mybir.AluOpType.mult

_B, _L, _D, _K = 4, 4096, 2048, 4
_N_CORES = 8
_SHARDS_PER_BATCH = _N_CORES // _B
_R = _L // _SHARDS_PER_BATCH      # 2048 output rows per core
_T = 4                            # time phases packed into partitions
_C = 128 // _T                    # channels per block (32)
_NB = _D // _C                    # blocks per core (64)
_TC = _R // _T + 1                # strip cols per block incl 1 halo col (513)
_NOUT = _R // _T                  # out cols per block (512)

# input chunk sizes in blocks: first small for fastest PE start
_IN_CHUNKS = (2, 4, 8, 12, 18, 20)
_ST_CHUNK = 8                     # blocks per DVE stat-build instruction
_ACT_BLKS = 4                     # blocks per activation (4*512 = 2048 cols)


def build_conv_kernel(nc: bass.Bass):
    NB, TC, NOUT = _NB, _TC, _NOUT
    xq_d = nc.dram_tensor("xq", [128, NB * TC], I8, kind="ExternalInput")
    wjm_d = nc.dram_tensor("wjm", [128, NB * _T], F16, kind="ExternalInput")
    wjc_d = nc.dram_tensor("wjc", [128, NB * _T], F16, kind="ExternalInput")
    cdg_d = nc.dram_tensor("cdg", [128, 128], F16, kind="ExternalInput")
    o_d = nc.dram_tensor("out", [128, NB * NOUT], F16, kind="ExternalOutput")

    with ExitStack() as ctx:
        tc = ctx.enter_context(tile.TileContext(nc))

        const_pool = ctx.enter_context(tc.tile_pool(name="const", bufs=1))
        xt_pool = ctx.enter_context(tc.tile_pool(name="xt", bufs=1))
        st_pool = ctx.enter_context(tc.tile_pool(name="st", bufs=1))
        ot_pool = ctx.enter_context(tc.tile_pool(name="ot", bufs=1))
        pc_pool = ctx.enter_context(tc.tile_pool(name="pc", bufs=2,
                                                 space="PSUM"))

        # tiny weight tensors first (sync HWDGE queue)
        wjm_t = const_pool.tile([128, NB * _T], F16)
        wjc_t = const_pool.tile([128, NB * _T], F16)
        cdg_t = const_pool.tile([128, 128], F16)
        nc.sync.dma_start(wjm_t, wjm_d[:, :])
        nc.sync.dma_start(wjc_t, wjc_d[:, :])
        nc.sync.dma_start(cdg_t, cdg_d[:, :])

        # input strip: int8 in DRAM, cast to fp16 by SWDGE during the DMA
        xt = xt_pool.tile([128, NB * TC], F16)
        b0 = 0
        for nb in _IN_CHUNKS:
            nc.gpsimd.dma_start(xt[:, b0 * TC:(b0 + nb) * TC],
                                xq_d[:, b0 * TC:(b0 + nb) * TC])
            b0 += nb
        assert b0 == NB

        # stationaries: stm/stc[p=(c,i), b*128 + (c'*4+j)] =
        #   (c==c') * w[32b+c, k] * s_x[32b+c], k=i-j+3 (main, i<=j)
        #   or k=i-j-1 (carry, i>j); zeros encoded in wjm/wjc from host.
        stm = st_pool.tile([128, NB * 128], F16)
        stc = st_pool.tile([128, NB * 128], F16)
        cd4 = cdg_t.rearrange("p (b c j) -> p b c j", b=1, j=_T)
        for s0 in range(0, NB, _ST_CHUNK):
            n = _ST_CHUNK
            cdb = cd4.broadcast_to([128, n, _C, _T])
            for st_t, wj_t in ((stm, wjm_t), (stc, wjc_t)):
                out4 = st_t.rearrange("p (b c j) -> p b c j",
                                      c=_C, j=_T)[:, s0:s0 + n]
                wj4 = wj_t.rearrange("p (b c j) -> p b c j",
                                     c=1, j=_T)[:, s0:s0 + n]
                nc.vector.tensor_tensor(
                    out4, cdb, wj4.broadcast_to([128, n, _C, _T]), MULT)

        ot = ot_pool.tile([128, NB * NOUT], F16)

        n_groups = NB // _ACT_BLKS
        for g in range(n_groups):
            pc = pc_pool.tile([128, _ACT_BLKS * NOUT], F32, tag="pc")
            for q in range(_ACT_BLKS):
                b = g * _ACT_BLKS + q
                xb = b * TC
                sl = pc[:, q * NOUT:(q + 1) * NOUT]
                nc.tensor.matmul(sl, stm[:, b * 128:(b + 1) * 128],
                                 xt[:, xb + 1:xb + 1 + NOUT],
                                 start=True, stop=False)
                nc.tensor.matmul(sl, stc[:, b * 128:(b + 1) * 128],
                                 xt[:, xb:xb + NOUT],
                                 start=False, stop=True)
            osl = slice(g * _ACT_BLKS * NOUT, (g + 1) * _ACT_BLKS * NOUT)
            nc.scalar.activation(ot[:, osl], pc, SILU)
            if g % 2 == 0:
                nc.sync.dma_start(o_d[:, osl], ot[:, osl])
            else:
                nc.scalar.dma_start(o_d[:, osl], ot[:, osl])

    return nc


# ---------------------------------------------------------------------------
# Entry point: full (unsharded) inputs -> full output, 8 NeuronCores.
# ---------------------------------------------------------------------------
from concourse.bass_utils import run_bass_kernel_spmd
import concourse.bacc as bacc

TRACE = False
LAST_EXEC_TIME_NS = None

_compiled_nc = None


def _get_nc():
    global _compiled_nc
    if _compiled_nc is None:
        nc = bacc.Bacc("TRN2", target_bir_lowering=False, debug=False)
        build_conv_kernel(nc)
        nc.compile()
        _compiled_nc = nc
    return _compiled_nc


def _host_pack(x_full: np.ndarray, w_full: np.ndarray):
    """Build the 8 per-core input maps (int8 strips + fp16 weight tiles)."""
    D, K, T, C, NB, TC = _D, _K, _T, _C, _NB, _TC
    ws = w_full.reshape(D, K)

    # (i, j) -> tap k lookup tables for main / carry stationaries
    in_maps = []
    scales = []
    for c in range(_N_CORES):
        b, s = divmod(c, _SHARDS_PER_BATCH)
        l0 = s * _R
        xs = x_full[b, l0:l0 + _R]                        # (R, D)
        halo = (x_full[b, l0 - (K - 1):l0] if s else
                np.zeros((K - 1, D), np.float32))         # (3, D)
        xall = np.concatenate([halo, xs], axis=0)         # (R+3, D)
        absmax = np.abs(xall).max(axis=0)
        sc = np.maximum(absmax, 1e-9) / 127.0             # (D,)
        q = np.rint(xall / sc).astype(np.int8)            # (R+3, D)

        # X2[d, 1:] = q.T ; col 0 (phase i=0 of halo block) never read
        X2 = np.zeros((D, _R + T), np.int8)
        X2[:, 1:] = q.T
        xq = (X2.reshape(NB, C, TC, T).transpose(0, 1, 3, 2)
              .reshape(NB, 128, TC).transpose(1, 0, 2)
              .reshape(128, NB * TC))

        # per-(p, block, j) weight tiles with the int8 scale folded in
        wsc = (ws * sc[:, None]).astype(np.float32)       # (D, K)
        w3 = wsc.reshape(NB, C, K)
        wjm = np.zeros((NB, C, T, T), np.float32)         # [b, c, i, j]
        wjc = np.zeros((NB, C, T, T), np.float32)
        for i in range(T):
            for j in range(T):
                if i <= j:
                    wjm[:, :, i, j] = w3[:, :, i - j + 3]
                else:
                    wjc[:, :, i, j] = w3[:, :, i - j - 1]
        wjm = (wjm.transpose(1, 2, 0, 3).reshape(128, NB * T)
               .astype(np.float16))
        wjc = (wjc.transpose(1, 2, 0, 3).reshape(128, NB * T)
               .astype(np.float16))

        cdg = np.kron(np.eye(C, dtype=np.float16),
                      np.ones((T, T), np.float16))        # (128, 128)

        in_maps.append({"xq": np.ascontiguousarray(xq),
                        "wjm": np.ascontiguousarray(wjm),
                        "wjc": np.ascontiguousarray(wjc),
                        "cdg": np.ascontiguousarray(cdg)})
        scales.append(sc)
    return in_maps, scales


def kernel(inputs: np.ndarray, weight: np.ndarray) -> np.ndarray:
    """inputs: (4, 4096, 2048) fp32; weight: (2048, 1, 4) fp32.

    Returns silu(causal_depthwise_conv1d(inputs, weight)): (4, 4096, 2048).
    """
    global LAST_EXEC_TIME_NS
    x_full = np.asarray(inputs, dtype=np.float32)
    w_full = np.asarray(weight, dtype=np.float32)
    assert x_full.shape == (_B, _L, _D), x_full.shape

    in_maps, _ = _host_pack(x_full, w_full)

    nc = _get_nc()
    res = run_bass_kernel_spmd(nc, in_maps, list(range(_N_CORES)),
                               trace=TRACE)
    LAST_EXEC_TIME_NS = res.exec_time_ns

    out = np.empty((_B, _L, _D), dtype=np.float32)
    for c in range(_N_CORES):
        b, s = divmod(c, _SHARDS_PER_BATCH)
        o = res.results[c]["out"]                          # (128, NB*512)
        Y = (o.reshape(_C, _T, _NB, _NOUT).transpose(2, 0, 3, 1)
             .reshape(_D, _R).astype(np.float32))          # (D, R)
        out[b, s * _R:(s + 1) * _R] = Y.T
    return out


# revision 3
# speedup vs baseline: 1.3512x; 1.0160x over previous
"""Causal depthwise conv1d (K=4) + SiLU on TRN2 — time-phase-packed fp16 design.

Per core (R=2048 out rows, D=2048 channels), v3 design:

 - Host packs each shard fp16 time-phase-major: partition p = (c, i) =
   32 channels x 4 time phases, column t = coarse time block of 4
   samples, one halo column per 32-channel block.
 - PE: per 32-channel block, TWO accumulating matmuls (main + carry)
   of 512 cols each compute all 4 taps: the stationary embeds the 4x4
   time-phase kernel per channel on the block diagonal. 64 blocks x 2
   x 512 cols = 65536 PE cycles ~ 27.5 us warm (vs 55 us for the
   4-matmul per-tap diagonal scheme).
 - Stationaries built on DVE: one broadcast multiply per 8-block chunk:
   chandiag (c==c' mask) x per-(p,j) weight tile (shipped, ~1KB).
 - ACT: Silu over [128, 2048] fp32 PSUM (4 banks) -> fp16 SBUF:
   16 instrs ~ 30 us total (vs 44 us at 512 cols).
 - DMA: input fp16 on the sync HWDGE ring (chunked FIFO, so input
   outranks output on that ring), output fp16 alternating scalar/sync.
   SDMA fabric is the binding resource: 16.8 MB SBUF-side bytes at
   ~400 GB/s ~ 42 us. (int8-in via SWDGE cast was tried: it halves HBM
   bytes but NOT SBUF-side fabric bytes, and costs an 8 us gpsimd
   bootstrap + late input -> slower. No engine can produce an int8
   output cheaply: ACT can't post-scale after Silu, DVE drops to 1x
   for 1-byte dtypes = 34 us. So fp16 both ways is optimal here.)
"""

from contextlib import ExitStack

import numpy as np

import concourse.bass as bass
import concourse.mybir as mybir
import concourse.tile as tile

F16 = mybir.dt.float16
F32 = mybir.dt.float32
SILU = mybir.ActivationFunctionType.Silu
MULT = mybir.AluOpType.mult

_B, _L, _D, _K = 4, 4096, 2048, 4
_N_CORES = 8
_SHARDS_PER_BATCH = _N_CORES // _B
_R = _L // _SHARDS_PER_BATCH      # 2048 output rows per core
_T = 4                            # time phases packed into partitions
_C = 128 // _T                    # channels per block (32)
_NB = _D // _C                    # blocks per core (64)
_TC = _R // _T + 1                # strip cols per block incl 1 halo col (513)
_NOUT = _R // _T                  # out cols per block (512)

# input chunk sizes in blocks: small early chunks for fast PE start and
# to keep PE fed at fine granularity (HAM stays warm)
_IN_CHUNKS = (4, 6, 8, 8, 8, 10, 10, 10)
_ST_CHUNK = 8                     # blocks per DVE stat-build instruction
_ACT_BLKS = 4                     # blocks per activation (4*512 = 2048 cols)


def build_conv_kernel(nc: bass.Bass):
    NB, TC, NOUT = _NB, _TC, _NOUT
    xs_d = nc.dram_tensor("xs", [128, NB * TC], F16, kind="ExternalInput")
    wjm_d = nc.dram_tensor("wjm", [128, NB * _T], F16, kind="ExternalInput")
    wjc_d = nc.dram_tensor("wjc", [128, NB * _T], F16, kind="ExternalInput")
    cdg_d = nc.dram_tensor("cdg", [128, 128], F16, kind="ExternalInput")
    o_d = nc.dram_tensor("out", [128, NB * NOUT], F16, kind="ExternalOutput")

    with ExitStack() as ctx:
        tc = ctx.enter_context(tile.TileContext(nc))

        const_pool = ctx.enter_context(tc.tile_pool(name="const", bufs=1))
        xt_pool = ctx.enter_context(tc.tile_pool(name="xt", bufs=1))
        st_pool = ctx.enter_context(tc.tile_pool(name="st", bufs=1))
        ot_pool = ctx.enter_context(tc.tile_pool(name="ot", bufs=1))
        pc_pool = ctx.enter_context(tc.tile_pool(name="pc", bufs=2,
                                                 space="PSUM"))

        # tiny weight tensors first on the sync HWDGE ring
        wjm_t = const_pool.tile([128, NB * _T], F16)
        wjc_t = const_pool.tile([128, NB * _T], F16)
        cdg_t = const_pool.tile([128, 128], F16)
        nc.sync.dma_start(wjm_t, wjm_d[:, :])
        nc.sync.dma_start(wjc_t, wjc_d[:, :])
        nc.sync.dma_start(cdg_t, cdg_d[:, :])

        # input strips, chunked FIFO on sync (ahead of any output there)
        xt = xt_pool.tile([128, NB * TC], F16)
        b0 = 0
        for nb in _IN_CHUNKS:
            nc.sync.dma_start(xt[:, b0 * TC:(b0 + nb) * TC],
                              xs_d[:, b0 * TC:(b0 + nb) * TC])
            b0 += nb
        assert b0 == NB

        # stationaries: stm/stc[p=(c,i), b*128 + (c'*4+j)] =
        #   (c==c') * w[32b+c, k], k=i-j+3 (main, i<=j) or k=i-j-1
        #   (carry, i>j); zeros encoded in wjm/wjc host-side.
        stm = st_pool.tile([128, NB * 128], F16)
        stc = st_pool.tile([128, NB * 128], F16)
        cd4 = cdg_t.rearrange("p (b c j) -> p b c j", b=1, j=_T)
        for s0 in range(0, NB, _ST_CHUNK):
            n = _ST_CHUNK
            cdb = cd4.broadcast_to([128, n, _C, _T])
            for st_t, wj_t in ((stm, wjm_t), (stc, wjc_t)):
                out4 = st_t.rearrange("p (b c j) -> p b c j",
                                      c=_C, j=_T)[:, s0:s0 + n]
                wj4 = wj_t.rearrange("p (b c j) -> p b c j",
                                     c=1, j=_T)[:, s0:s0 + n]
                nc.vector.tensor_tensor(
                    out4, cdb, wj4.broadcast_to([128, n, _C, _T]), MULT)

        ot = ot_pool.tile([128, NB * NOUT], F16)

        n_groups = NB // _ACT_BLKS
        for g in range(n_groups):
            pc = pc_pool.tile([128, _ACT_BLKS * NOUT], F32, tag="pc")
            for q in range(_ACT_BLKS):
                b = g * _ACT_BLKS + q
                xb = b * TC
                sl = pc[:, q * NOUT:(q + 1) * NOUT]
                nc.tensor.matmul(sl, stm[:, b * 128:(b + 1) * 128],
                                 xt[:, xb + 1:xb + 1 + NOUT],
                                 start=True, stop=False)
                nc.tensor.matmul(sl, stc[:, b * 128:(b + 1) * 128],
                                 xt[:, xb:xb + NOUT],
                                 start=False, stop=True)
            osl = slice(g * _ACT_BLKS * NOUT, (g + 1) * _ACT_BLKS * NOUT)
            nc.scalar.activation(ot[:, osl], pc, SILU)
            # outputs: scalar ring + sync ring (sync ones queue behind the
            # remaining input chunks -> input keeps priority early, both
            # rings drain outputs at the tail)
            if g % 2 == 0:
                nc.scalar.dma_start(o_d[:, osl], ot[:, osl])
            else:
                nc.sync.dma_start(o_d[:, osl], ot[:, osl])

    return nc


# ---------------------------------------------------------------------------
# Entry point: full (unsharded) inputs -> full output, 8 NeuronCores.
# ---------------------------------------------------------------------------
from concourse.bass_utils import run_bass_kernel_spmd
import concourse.bacc as bacc

TRACE = False
LAST_EXEC_TIME_NS = None

_compiled_nc = None


def _get_nc():
    global _compiled_nc
    if _compiled_nc is None:
        nc = bacc.Bacc("TRN2", target_bir_lowering=False, debug=False)
        build_conv_kernel(nc)
        nc.compile()
        _compiled_nc = nc
    return _compiled_nc


def _host_pack(x_full: np.ndarray, w_full: np.ndarray):
    """Build the 8 per-core input maps (fp16 strips + weight tiles)."""
    D, K, T, C, NB, TC = _D, _K, _T, _C, _NB, _TC
    ws = w_full.reshape(D, K)

    # per-(p=(c,i), block, j) weight tiles: main (i<=j, k=i-j+3) and
    # carry (i>j, k=i-j-1); shared by all cores
    w3 = ws.reshape(NB, C, K)
    wjm = np.zeros((NB, C, T, T), np.float32)             # [b, c, i, j]
    wjc = np.zeros((NB, C, T, T), np.float32)
    for i in range(T):
        for j in range(T):
            if i <= j:
                wjm[:, :, i, j] = w3[:, :, i - j + 3]
            else:
                wjc[:, :, i, j] = w3[:, :, i - j - 1]
    wjm = (wjm.transpose(1, 2, 0, 3).reshape(128, NB * T)
           .astype(np.float16))
    wjc = (wjc.transpose(1, 2, 0, 3).reshape(128, NB * T)
           .astype(np.float16))
    cdg = np.kron(np.eye(C, dtype=np.float16),
                  np.ones((T, T), np.float16))            # (128, 128)

    in_maps = []
    for c in range(_N_CORES):
        b, s = divmod(c, _SHARDS_PER_BATCH)
        l0 = s * _R
        # X2[d, 1:] = x[l0-3 .. l0+R-1].T ; col 0 (phase 0 of the halo
        # block) is never read by the carry stationary (its i=0 rows are 0)
        X2 = np.zeros((D, _R + T), np.float16)
        X2[:, 4:] = x_full[b, l0:l0 + _R].astype(np.float16).T
        if s:
            X2[:, 1:4] = x_full[b, l0 - 3:l0].astype(np.float16).T
        xs = (X2.reshape(NB, C, TC, T).transpose(0, 1, 3, 2)
              .reshape(NB, 128, TC).transpose(1, 0, 2)
              .reshape(128, NB * TC))
        in_maps.append({"xs": np.ascontiguousarray(xs),
                        "wjm": wjm, "wjc": wjc, "cdg": cdg})
    return in_maps


def kernel(inputs: np.ndarray, weight: np.ndarray) -> np.ndarray:
    """inputs: (4, 4096, 2048) fp32; weight: (2048, 1, 4) fp32.

    Returns silu(causal_depthwise_conv1d(inputs, weight)): (4, 4096, 2048).
    """
    global LAST_EXEC_TIME_NS
    x_full = np.asarray(inputs, dtype=np.float32)
    w_full = np.asarray(weight, dtype=np.float32)
    assert x_full.shape == (_B, _L, _D), x_full.shape

    in_maps = _host_pack(x_full, w_full)

    nc = _get_nc()
    res = run_bass_kernel_spmd(nc, in_maps, list(range(_N_CORES)),
                               trace=TRACE)
    LAST_EXEC_TIME_NS = res.exec_time_ns

    out = np.empty((_B, _L, _D), dtype=np.float32)
    for c in range(_N_CORES):
        b, s = divmod(c, _SHARDS_PER_BATCH)
        o = res.results[c]["out"]                          # (128, NB*512)
        Y = (o.reshape(_C, _T, _NB, _NOUT).transpose(2, 0, 3, 1)
             .reshape(_D, _R).astype(np.float32))          # (D, R)
        out[b, s * _R:(s + 1) * _R] = Y.T
    return out
